# revision 11
# baseline (speedup 1.0000x reference)
# Pairwise Euclidean distance kernel for Trainium2 (Bass/Tile).
#
# Input : coordinates_batch [8, 2048, 3] f32
# Output: [8, 2048, 2048] f32, out[b,i,j] = ||c[b,i] - c[b,j]||
#
# Sharding: data-parallel over batch -- one batch element per NeuronCore (8 cores).
#
# Per-core algorithm: dist^2(i,j) = |ci|^2 + |cj|^2 - 2 ci.cj.
# The cross term and |cj|^2 ride a single K=21 bf16 matmul per output tile:
# each fp32 row of the augmented matrices is 3-way bf16-split (h + m + l) and
# the product keeps the 6 significant digit-pair groups
#   (h,h),(m,h),(h,m),(l,h),(h,l),(m,m)
# stacked along K (3 coord rows per group = 18 rows) plus 3 rows (1 x n2_{h,m,l})
# for |cj|^2 -- residual ~2^-27, i.e. fp32-grade accuracy at bf16 streaming
# speed (1 cycle/column on the PE instead of 8 for fp32 x fp32).
# |ci|^2 is added EXACTLY in fp32: the vector engine drains PSUM with
# row = max(psum, -n2_i) and the scalar engine computes sqrt(row + n2_i)
# (per-partition bias AP), i.e. sqrt(max(dist^2, 0)) overall.
# The diagonal is zeroed host-side during the gather (translation glue),
# keeping the device graph to 4 engines (PE, DVE, ACT, Sync/DMA).
#
# Pipeline is chunked at [128, 1024] (2 matmuls -> 1 clamp -> 1 sqrt -> 1 DMA
# of 512 KiB) so the output-DMA stream -- the binding resource at ~400 GB/s --
# starts as early as possible. Input loads are split so the first matmul only
# waits for ~27 KB.

import numpy as np

B, N, D = 8, 2048, 3
K = 21           # stacked bf16-split contraction dim
P = 128          # output row tile (partition dim)
FT = 512         # matmul free-dim chunk (one PSUM bank of f32)
CH = 1024        # pipeline chunk (2 PSUM banks)
NI = N // P      # 16 row tiles
NC = N // CH     # 2 chunks per row tile

_cached_nc = None


def _build_nc():
    global _cached_nc
    if _cached_nc is not None:
        return _cached_nc

    import concourse.bacc as bacc
    import concourse.mybir as mybir
    import concourse.tile as tile

    nc = bacc.Bacc("TRN2", target_bir_lowering=False, debug=False)
    bf16 = mybir.dt.bfloat16
    f32 = mybir.dt.float32

    Ld = nc.dram_tensor("lhs", [K, N], bf16, kind="ExternalInput")
    Rd = nc.dram_tensor("rhs", [K, N], bf16, kind="ExternalInput")
    Nd = nc.dram_tensor("n2b", [P, 2 * NI], f32, kind="ExternalInput")  # [n2 | -n2]
    out = nc.dram_tensor("out", [N, N], f32, kind="ExternalOutput")

    with tile.TileContext(nc) as tc:
        with (
            tc.tile_pool(name="singles", bufs=1) as singles,
            tc.tile_pool(name="rows", bufs=12) as rows,
            tc.tile_pool(name="psum", bufs=4, space="PSUM") as psum,
        ):
            L = singles.tile([K, N], bf16)
            R = singles.tile([K, N], bf16)
            n2b = singles.tile([P, 2 * NI], f32)

            # Warm up the ACT sqrt table with a no-dependency dummy op so the
            # 1.3 us ACT_TABLE_LOAD overlaps boot instead of the first row.
            scratch = singles.tile([1, 2], f32)
            nc.gpsimd.memset(scratch, 1.0)
            nc.scalar.sqrt(scratch, scratch)

            # Head loads first on Sync's HWDGE (fast issue): exactly what the
            # first row-tile needs. Bulk loads ride gpsimd's SW DGE -- slower
            # issue, but they are only needed a few microseconds later.
            nc.sync.dma_start(out=L[:, 0:P], in_=Ld.ap()[:, 0:P])
            nc.sync.dma_start(out=R[:, 0:CH], in_=Rd.ap()[:, 0:CH])
            nc.sync.dma_start(out=n2b, in_=Nd.ap())
            nc.gpsimd.dma_start(out=L[:, P:], in_=Ld.ap()[:, P:])
            nc.gpsimd.dma_start(out=R[:, CH:], in_=Rd.ap()[:, CH:])

            for it in range(NI):
                lhsT = L[:, it * P : (it + 1) * P]
                # Row 0 runs at [128, 512] granularity to put the first output
                # chunk on the DMA wire as early as possible; later rows use
                # [128, 1024] chunks (lower per-op overhead on DVE/ACT).
                chw = CH
                for jc in range(N // chw):
                    ps = psum.tile([P, chw], f32, tag="ps")
                    for sub in range(chw // FT):
                        lo = sub * FT
                        nc.tensor.matmul(
                            ps[:, lo : lo + FT],
                            lhsT=lhsT,
                            rhs=R[:, jc * chw + lo : jc * chw + lo + FT],
                            start=True,
                            stop=True,
                        )
                    chunk = rows.tile([P, chw], f32, tag="chunk")
                    # chunk = max(psum, -n2_i); sqrt(chunk + n2_i) on ACT ==
                    # sqrt(max(dist^2, 0)) with n2_i exact in fp32.
                    nc.vector.tensor_scalar_max(chunk, ps, n2b[:, NI + it : NI + it + 1])
                    nc.scalar.activation(
                        chunk, chunk, mybir.ActivationFunctionType.Sqrt,
                        bias=n2b[:, it : it + 1], scale=1.0,
                    )
                    nc.sync.dma_start(
                        out=out[it * P : (it + 1) * P, jc * chw : (jc + 1) * chw],
                        in_=chunk,
                    )

    nc.compile()
    _cached_nc = nc
    return nc


def _augment(x: np.ndarray):
    """x: [B, N, 3] f32 -> (lhsT [B,21,N] bf16, rhs [B,21,N] bf16, n2b [B,128,32] f32)."""
    import ml_dtypes

    bf = ml_dtypes.bfloat16

    def split3(a):
        h = a.astype(bf).astype(np.float32)
        r = a - h
        m = r.astype(bf).astype(np.float32)
        l = (r - m).astype(bf).astype(np.float32)
        return h, m, l

    nb = x.shape[0]
    xt = np.transpose(x, (0, 2, 1)).astype(np.float32)           # [B,3,N]
    n2 = np.sum(x.astype(np.float64) ** 2, axis=2).astype(np.float32)  # [B,N]
    m2 = (-2.0 * x.astype(np.float64)).astype(np.float32).transpose(0, 2, 1)  # [B,3,N]

    ch, cm, cl = split3(xt)
    mh, mm, ml = split3(m2)
    nh, nm, nl = split3(n2[:, None, :])
    one = np.ones((nb, 1, x.shape[1]), np.float32)

    lhsT = np.concatenate([ch, cm, ch, cl, ch, cm, one, one, one], 1)  # [B,21,N]
    rhs = np.concatenate([mh, mh, mm, mh, ml, mm, nh, nm, nl], 1)      # [B,21,N]
    n2t = np.transpose(n2.reshape(nb, NI, P), (0, 2, 1))               # [B,128,16]
    n2b = np.concatenate([n2t, -n2t], axis=2).astype(np.float32)       # [B,128,32]
    return (
        np.ascontiguousarray(lhsT.astype(bf)),
        np.ascontiguousarray(rhs.astype(bf)),
        np.ascontiguousarray(n2b),
    )


def run(coordinates_batch: np.ndarray, trace: bool = False):
    """Run on 8 NeuronCores; returns (output [8,2048,2048] f32, BassKernelResults)."""
    from concourse.bass_utils import run_bass_kernel_spmd

    nc = _build_nc()
    x = np.ascontiguousarray(np.asarray(coordinates_batch), dtype=np.float32)
    assert x.shape == (B, N, D), x.shape
    lhsT, rhs, n2b = _augment(x)
    in_maps = [{"lhs": lhsT[b], "rhs": rhs[b], "n2b": n2b[b]} for b in range(B)]
    res = run_bass_kernel_spmd(nc, in_maps, core_ids=list(range(B)), trace=trace)
    out = np.stack([r["out"] for r in res.results], axis=0)
    # exact-zero diagonal (dist(i,i) == 0), part of the gather/unshard glue
    out.reshape(B, -1)[:, :: N + 1] = 0.0
    return out, res


def kernel(coordinates_batch: np.ndarray) -> np.ndarray:
    out, _ = run(coordinates_batch, trace=False)
    return out
